# revision 11
# baseline (speedup 1.0000x reference)
"""Multi-head distance (attention) layer on 8 TRN2 NeuronCores — v2.

Sharding: data-parallel over batch, B=8 -> one batch element per core.

Key differences vs v1 (99.7-118us):
  - x.T and (x+pe).T are computed on the host and DMA'd directly: no PE
    transposes, no DVE pos-enc adds, shorter startup critical path.
  - S matmuls are ROW-TILED pairs: head 2u on PE rows 0-63, head 2u+1 on
    rows 64-127 (K=64 each, tile_position auto-derived from base
    partitions). The two streams run concurrently -> ~2x S throughput,
    and no kTz zero-padding/memsets.
  - exp runs on [128, 3, 512] PSUM groups (1536 els/lane per ACTIVATE)
    instead of 1024 -> fewer ACT calls, less per-call overhead.
  - O is computed v-stationary: lhsT = v_aug[mc] [128, 65], rhs = e
    chunk [128, 512], accumulating O^T[65, l] over mc in PSUM. The 65th
    row is the softmax denominator Z (ones column of v_aug). O^T is
    DMA'd straight from PSUM to DRAM; the host divides by Z, transposes,
    and adds repeat(bv, 64). This removes ~512 LDWEIGHTS (was the PE
    bottleneck) and all DVE normalize/drain work.
  - bq is added during the qT PSUM drain (per-partition scalar on DVE);
    bk only shifts scores by a per-column constant (softmax-invariant)
    so it is dropped.

PSUM (8 banks): sA, sB [128, 3, 512] f32 (banks 0-5, exp groups),
oacc [65, 512] (bank 6, O^T accumulator), pp [128, 512] (bank 7, QKV
projection scratch).
"""

import numpy as np

import concourse.bass as bass
import concourse.mybir as mybir
import concourse.tile as tile
from concourse import bacc
from concourse.bass_utils import run_bass_kernel_spmd

B, L, D = 8, 1024, 256
H, HD = 8, 64
J = H * HD  # 512
TEMPERATURE = 10000.0

f32 = mybir.dt.float32
f16 = mybir.dt.float16

_CACHE = {}
LAST_RESULT = None
TRACE = False


def _emit(tc, aps):
    nc = tc.nc
    Exp = mybir.ActivationFunctionType.Exp
    xTd, xpd, wqd, wkd, wvd, bqd, oTd = (
        aps["xT"], aps["xpeT"], aps["wq"], aps["wk"], aps["wv"], aps["bqc"],
        aps["oT"],
    )
    xTr = xTd.rearrange("(t p) l -> t p l", p=128)    # [2, 128, 1024]
    xpr = xpd.rearrange("(t p) l -> t p l", p=128)
    wqr = wqd.rearrange("(t p) j -> t p j", p=128)    # [2, 128, 512]
    wkr = wkd.rearrange("(t p) j -> t p j", p=128)
    wvr = wvd.rearrange("(t p) j -> t p j", p=128)

    import contextlib
    ctx = contextlib.ExitStack()
    persist = ctx.enter_context(tc.tile_pool(name="persist", bufs=1))
    epool = ctx.enter_context(tc.tile_pool(name="epool", bufs=24))
    pspool = ctx.enter_context(tc.tile_pool(name="ps", bufs=1, space="PSUM"))

    # --- ACT exp-table preload (off the critical path) ---
    sc_in = persist.tile([128, 8], f32, name="sc_in")
    sc_out = persist.tile([128, 8], f32, name="sc_out")
    nc.vector.memset(sc_in[:], 0.0)
    nc.scalar.activation(sc_out[:], sc_in[:], Exp)

    # --- SBUF ---
    xpe_sb = [persist.tile([128, 1024], f16, name=f"xpe{t}") for t in range(2)]
    xT_sb = [persist.tile([128, 1024], f16, name=f"xT{t}") for t in range(2)]
    w_sb = {
        w: [persist.tile([128, 512], f16, name=f"{w}{t}") for t in range(2)]
        for w in ("wq", "wk", "wv")
    }
    bq_sb = persist.tile([128, 4], f32, name="bq_sb")
    kT = [persist.tile([128, 1024], f16, name=f"kT{u}") for u in range(4)]
    qT = [persist.tile([128, 1024], f16, name=f"qT{u}") for u in range(4)]
    v_sb = [persist.tile([128, 8, 65], f16, name=f"v{m}") for m in range(8)]

    # --- PSUM: 3+3 exp-group banks, 1 O bank, 1 projection bank ---
    sgrp = [pspool.tile([128, 3, 512], f32, name="sA")]
    pp = pspool.tile([128, 512], f32, name="pp")
    sgrp.append(pspool.tile([128, 3, 512], f32, name="sB"))
    oacc = pspool.tile([65, 512], f32, name="oacc")

    # --- input DMAs: three parallel queues (sync->Q1, gpsimd->Q0,
    # scalar->Q10, each ~75 GB/s, ~740ns issue). Critical path to the
    # first S chunk is K(0,0)+Q(0,0): wk/wq j-cols 0:128 + xpe l-half 0,
    # 384KB split three ways. Everything else queues behind. ---
    # critical wave on the two HWDGE queues (sync, scalar) — SWDGE
    # (gpsimd) transfers start late and run slow, keep it off the
    # startup path
    nc.sync.dma_start(out=xpe_sb[0][:, 0:512], in_=xpr[0][:, 0:512])
    nc.sync.dma_start(out=w_sb["wq"][0][:, 0:128], in_=wqr[0][:, 0:128])
    nc.sync.dma_start(out=w_sb["wk"][0][:, 0:128], in_=wkr[0][:, 0:128])
    nc.scalar.dma_start(out=bq_sb[:], in_=bqd[:, :])
    nc.scalar.dma_start(out=xpe_sb[1][:, 0:512], in_=xpr[1][:, 0:512])
    nc.scalar.dma_start(out=w_sb["wq"][1][:, 0:128], in_=wqr[1][:, 0:128])
    nc.scalar.dma_start(out=w_sb["wk"][1][:, 0:128], in_=wkr[1][:, 0:128])
    # second wave: xpe h1 (K/Q(0,1)), then w rests (K/Q(1..3) start
    # early in pair 0), wv+xT (V pieces run late in pair 0)
    nc.sync.dma_start(out=xpe_sb[0][:, 512:1024], in_=xpr[0][:, 512:1024])
    nc.scalar.dma_start(out=xpe_sb[1][:, 512:1024], in_=xpr[1][:, 512:1024])
    nc.sync.dma_start(out=w_sb["wk"][0][:, 128:512], in_=wkr[0][:, 128:512])
    nc.scalar.dma_start(out=w_sb["wk"][1][:, 128:512], in_=wkr[1][:, 128:512])
    nc.sync.dma_start(out=w_sb["wq"][0][:, 128:512], in_=wqr[0][:, 128:512])
    nc.scalar.dma_start(out=w_sb["wq"][1][:, 128:512], in_=wqr[1][:, 128:512])
    nc.gpsimd.dma_start(out=w_sb["wv"][0][:], in_=wvr[0])
    nc.gpsimd.dma_start(out=w_sb["wv"][1][:], in_=wvr[1])
    nc.gpsimd.dma_start(out=xT_sb[0][:], in_=xTr[0])
    nc.sync.dma_start(out=xT_sb[1][:], in_=xTr[1])
    # ones columns of v_aug (gpsimd: SBUF-only op, keeps DVE free)
    for m in range(8):
        nc.gpsimd.memset(v_sb[m][:, :, 64:65], 1.0)

    # --- projections (PSUM bank 3, DVE drains) ---
    def kq_piece(u, which, l2, scratch=None):
        dst = pp[:] if scratch is None else scratch
        wname = "wq" if which == "q" else "wk"
        for c2 in range(2):
            nc.tensor.matmul(
                dst,
                lhsT=w_sb[wname][c2][:, u * 128:(u + 1) * 128],
                rhs=xpe_sb[c2][:, l2 * 512:(l2 + 1) * 512],
                start=(c2 == 0),
                stop=(c2 == 1),
            )
        dsl = slice(l2 * 512, (l2 + 1) * 512)
        if which == "q":
            nc.vector.tensor_scalar_add(qT[u][:, dsl], dst, bq_sb[:, u:u + 1])
        else:
            nc.vector.tensor_copy(kT[u][:, dsl], dst)

    def v_piece(m):
        for c2 in range(2):
            nc.tensor.matmul(
                pp[:],
                lhsT=xT_sb[c2][:, m * 128:(m + 1) * 128],
                rhs=w_sb["wv"][c2][:],
                start=(c2 == 0),
                stop=(c2 == 1),
            )
        nc.vector.tensor_copy(
            v_sb[m][:, :, 0:64], pp[:].rearrange("p (h d) -> p h d", h=8)
        )

    # --- S chunks + grouped exp ---
    epos = {}  # (h, mc, l2) -> (e_tile, chunk_idx)
    st = {"g": 0, "c": 0, "gi": 0, "warm": 0, "keys": []}

    # exp(0.125*s) on DVE via the Schraudolph bit trick: the fp32->u16
    # convert truncates, so the +0.5 recentres; u16 wraps (not
    # saturates) below i=0, safe while |s| < 82 (data: |s|max ~ 56).
    ALPHA = float(0.125 * 1024.0 / np.log(2.0))
    BETA = float(1024.0 * (15.0 - 0.05) + 0.5)
    u16 = mybir.dt.uint16

    def flush_exp():
        n = st["c"]
        if n == 0:
            return
        e = epool.tile([128, 3, 512], f16, tag="e", name="e")
        if st["gi"] % 4 == 2 and n == 3:
            nc.vector.tensor_scalar(
                e[:, 0:n, :].bitcast(u16), sgrp[st["g"]][:, 0:n, :],
                ALPHA, BETA, mybir.AluOpType.mult, mybir.AluOpType.add,
            )
        else:
            nc.scalar.activation(
                e[:, 0:n, :], sgrp[st["g"]][:, 0:n, :], Exp,
                scale=float(HD) ** -0.5
            )
        for i, key in enumerate(st["keys"]):
            epos[key] = (e, i)
        st["g"] ^= 1
        st["gi"] += 1
        st["c"] = 0
        st["keys"] = []

    def s_chunk(h, mc, l2):
        u, half = h // 2, (h % 2) * 64
        dst = sgrp[st["g"]][:, st["c"], :]
        nc.tensor.matmul(
            dst,
            lhsT=kT[u][half:half + 64, mc * 128:(mc + 1) * 128],
            rhs=qT[u][half:half + 64, l2 * 512:(l2 + 1) * 512],
            start=True,
            stop=True,
        )
        st["keys"].append((h, mc, l2))
        st["c"] += 1
        if st["warm"] > 0:
            st["warm"] -= 1
            flush_exp()
        elif st["c"] == 3:
            flush_exp()

    # --- O: v-stationary accumulation of O^T into oacc; DVE drains to
    # SBUF (DMA has no PSUM read path), then DMA out ---
    ODMA = [nc.sync, nc.gpsimd, nc.sync, nc.gpsimd,
            nc.sync, nc.gpsimd, nc.sync, nc.gpsimd]
    opool = ctx.enter_context(tc.tile_pool(name="opool", bufs=4))

    def o_mm(h, l2, mc, acc=None):
        e, ci = epos[(h, mc, l2)]
        nc.tensor.matmul(
            oacc[:] if acc is None else acc,
            lhsT=v_sb[mc][:, h, :],
            rhs=e[:, ci, :],
            start=(mc == 0),
            stop=(mc == 7),
        )

    def o_dma(h, l2, acc=None):
        # split the write-back across two queues (three in the tail,
        # when ACT has gone idle) so the final DMAs don't serialize
        o_sb = opool.tile([65, 512], f32, tag="o", name="o_sb")
        nc.vector.tensor_copy(o_sb[:], oacc[:] if acc is None else acc)
        engs = (nc.sync, nc.scalar) if h >= 5 else (nc.sync, nc.gpsimd)
        n = len(engs)
        w = 512 // n
        for i, eng in enumerate(engs):
            sl = slice(i * w, 512 if i == n - 1 else (i + 1) * w)
            eng.dma_start(out=oTd[h, l2][:, sl], in_=o_sb[:, sl])

    # ---------------- schedule ----------------
    # startup: K and Q on different PSUM banks so their MMs/drains
    # overlap instead of serializing on one scratch bank
    kq_piece(0, "k", 0)
    kq_piece(0, "q", 0, scratch=sgrp[1][:, 0, :])
    st["warm"] = 6

    def s_order_pair(u):
        hA, hB = 2 * u, 2 * u + 1
        seq = []
        for mc in range(8):
            seq += [(hA, mc, 0), (hB, mc, 0), (hA, mc, 1), (hB, mc, 1)]
        return seq

    # Driver: emit S chunks in groups of 3 FIRST (the exp fires at the
    # 3rd chunk), then that group's share of O/proj/V work. Keeping the
    # S chunks at the head of each in-order PE queue segment keeps ACT
    # fed; the extras fill the exp window behind them.
    def run_section(chunks, extras, flush_after=True, pace=None):
        ex = list(extras)
        n = len(chunks)
        emitted = 0
        for i, chk in enumerate(chunks):
            s_chunk(*chk)
            if pace is None:
                target = int(len(extras) * (i + 1) / (n + 1))
            else:
                target = max([v for t, v in pace.items() if t <= i + 1] or [0])
            while emitted < target and ex:
                ex.pop(0)()
                emitted += 1
        if flush_after:
            flush_exp()
        while ex:
            ex.pop(0)()

    # pair 0: chunk order staged by DMA arrival: mc<4 l2q=0 needs only
    # K(0,0)+Q(0,0); l2q=1 needs Q(0,1) (xpe h1); mc>=4 needs K(0,1).
    seq0 = []
    for mc in range(4):
        seq0 += [(0, mc, 0), (1, mc, 0)]
    for mc in range(4):
        seq0 += [(0, mc, 1), (1, mc, 1)]
    for mc in range(4, 8):
        seq0 += [(0, mc, 0), (1, mc, 0)]
    for mc in range(4, 8):
        seq0 += [(0, mc, 1), (1, mc, 1)]
    # K(0,1)/Q(0,1) must precede chunk 8 (first l2q=1). V pieces are NOT
    # here: xT arrives late on the slow SWDGE queue, and a stalled V MM
    # would block the in-order PE queue; V runs in pair 1, just ahead of
    # its O consumers.
    extras0 = [
        lambda: kq_piece(0, "k", 1), lambda: kq_piece(0, "q", 1),
        lambda: kq_piece(1, "k", 0), lambda: kq_piece(1, "k", 1),
        lambda: kq_piece(1, "q", 0), lambda: kq_piece(1, "q", 1),
    ]
    run_section(seq0, extras0,
                pace={4: 2, 14: 3, 18: 4, 23: 5, 27: 6})

    # O work is allocated to sections one series later than the naive
    # split so each section's PE load stays under the ACT exp pace
    # (pair 1 also carries the 8 V pieces).
    def o_series_thunks(oh, ol2):
        t = [(lambda a, b, c: (lambda: o_mm(a, b, c)))(oh, ol2, mi) for mi in range(8)]
        t.append((lambda a, b: (lambda: o_dma(a, b)))(oh, ol2))
        return t

    def kq_thunk(u, w, l2):
        return (lambda a, b, c: (lambda: kq_piece(a, b, c)))(u, w, l2)

    # pair 1: V pieces (2 ahead of their O consumers) + O(h0) + O(h1,0)
    extras = [(lambda m: (lambda: v_piece(m)))(m) for m in (0, 1)]
    s00 = o_series_thunks(0, 0)
    for mi in range(6):
        extras.append((lambda m: (lambda: v_piece(m)))(mi + 2))
        extras.append(s00[mi])
    extras += s00[6:]
    extras += [kq_thunk(2, "k", 0), kq_thunk(2, "k", 1)]
    extras += o_series_thunks(0, 1)
    extras += [kq_thunk(2, "q", 0), kq_thunk(2, "q", 1)]
    extras += o_series_thunks(1, 0)
    run_section(s_order_pair(1), extras)

    # pair 2: O(h1,1) + O(h2) + O(h3) + K/Q(3) — pair 2 has PE slack,
    # so it absorbs all of head 3's output work, relieving pair 3
    extras = o_series_thunks(1, 1)
    extras += [kq_thunk(3, "k", 0), kq_thunk(3, "k", 1)]
    extras += o_series_thunks(2, 0)
    extras += [kq_thunk(3, "q", 0), kq_thunk(3, "q", 1)]
    extras += o_series_thunks(2, 1) + o_series_thunks(3, 0) + o_series_thunks(3, 1)
    run_section(s_order_pair(2), extras)

    # pair 3: head-sequential S so O(hA) can overlap S(hB)
    hA, hB = 6, 7
    seqA = [(hA, mc, l2) for mc in range(8) for l2 in range(2)]
    seqB = [(hB, mc, l2) for mc in range(8) for l2 in range(2)]
    extras = o_series_thunks(4, 0) + o_series_thunks(4, 1) + o_series_thunks(5, 0)
    run_section(seqA, extras)  # flush: hA fully exp'd before O(hA)
    ppo = pp[0:65, :]
    extras = (o_series_thunks(5, 1)
              + o_series_thunks(hA, 0) + o_series_thunks(hA, 1))
    # O(hB): both l2-series on separate banks, interleaved mc-by-mc so
    # only the last couple of MMs trail the final exp
    for mc in range(8):
        extras.append((lambda m: (lambda: o_mm(hB, 0, m, acc=oacc[:])))(mc))
        extras.append((lambda m: (lambda: o_mm(hB, 1, m, acc=ppo)))(mc))
    extras.append(lambda: o_dma(hB, 0, acc=oacc[:]))
    extras.append(lambda: o_dma(hB, 1, acc=ppo))
    run_section(seqB, extras)
    ctx.close()


def _build(debug=False):
    key = ("nc", debug)
    if key in _CACHE:
        return _CACHE[key]
    nc = bacc.Bacc("TRN2", target_bir_lowering=False, debug=debug, num_devices=8)
    aps = {
        "xT": nc.dram_tensor("xT", [D, L], f16, kind="ExternalInput").ap(),
        "xpeT": nc.dram_tensor("xpeT", [D, L], f16, kind="ExternalInput").ap(),
        "wq": nc.dram_tensor("wq", [D, J], f16, kind="ExternalInput").ap(),
        "wk": nc.dram_tensor("wk", [D, J], f16, kind="ExternalInput").ap(),
        "wv": nc.dram_tensor("wv", [D, J], f16, kind="ExternalInput").ap(),
        "bqc": nc.dram_tensor("bqc", [128, 4], f32, kind="ExternalInput").ap(),
        "oT": nc.dram_tensor("oT", [H, 2, 65, 512], f32, kind="ExternalOutput").ap(),
    }
    with tile.TileContext(nc) as tc:
        _emit(tc, aps)
    nc.compile()
    _CACHE[key] = nc
    return nc


def _pe():
    embed = np.arange(L, dtype=np.float32)
    dim_t = np.arange(D, dtype=np.float32)
    dim_t = (np.float32(TEMPERATURE) ** (2.0 * np.floor(dim_t / 2.0) / np.float32(D))).astype(np.float32)
    pos = embed[:, None] / dim_t
    return np.stack(
        [np.sin(pos[:, 0::2]), np.cos(pos[:, 1::2])], axis=2
    ).reshape(L, D).astype(np.float32)


def make_in_maps(inputs):
    x = np.asarray(inputs["x"], dtype=np.float32)
    wq = np.ascontiguousarray(np.asarray(inputs["Wq"], np.float32).astype(np.float16))
    wk = np.ascontiguousarray(np.asarray(inputs["Wk"], np.float32).astype(np.float16))
    wv = np.ascontiguousarray(np.asarray(inputs["Wv"], np.float32).astype(np.float16))
    bq = np.asarray(inputs["bq"], dtype=np.float32)
    bqc = np.ascontiguousarray(np.repeat(bq, HD).reshape(4, 128).T)
    pe = _pe()
    base = {"wq": wq, "wk": wk, "wv": wv, "bqc": bqc}
    in_maps = []
    for b in range(B):
        xT = np.ascontiguousarray(x[b].T.astype(np.float16))
        xpeT = np.ascontiguousarray((x[b] + pe).T.astype(np.float16))
        in_maps.append({**base, "xT": xT, "xpeT": xpeT})
    return in_maps


def finish(oT_list, bv):
    outs = []
    for oT in oT_list:  # each [8, 2, 65, 512]
        N = oT[:, :, :64, :] / oT[:, :, 64:65, :]      # [8, 2, 64, 512]
        outs.append(N.transpose(1, 3, 0, 2).reshape(L, J))
    out = np.stack(outs).astype(np.float32)
    out += np.repeat(np.asarray(bv, np.float32), HD)[None, None, :]
    return out


def kernel(**inputs):
    global LAST_RESULT
    nc = _build()
    in_maps = make_in_maps(inputs)
    res = run_bass_kernel_spmd(nc, in_maps, core_ids=list(range(B)), trace=TRACE)
    LAST_RESULT = res
    return finish([res.results[b]["oT"] for b in range(B)], inputs["bv"])


# revision 13
# speedup vs baseline: 1.0019x; 1.0019x over previous
"""Multi-head distance (attention) layer on 8 TRN2 NeuronCores — v2.

Sharding: data-parallel over batch, B=8 -> one batch element per core.

Key differences vs v1 (99.7-118us):
  - x.T and (x+pe).T are computed on the host and DMA'd directly: no PE
    transposes, no DVE pos-enc adds, shorter startup critical path.
  - S matmuls are ROW-TILED pairs: head 2u on PE rows 0-63, head 2u+1 on
    rows 64-127 (K=64 each, tile_position auto-derived from base
    partitions). The two streams run concurrently -> ~2x S throughput,
    and no kTz zero-padding/memsets.
  - exp runs on [128, 3, 512] PSUM groups (1536 els/lane per ACTIVATE)
    instead of 1024 -> fewer ACT calls, less per-call overhead.
  - O is computed v-stationary: lhsT = v_aug[mc] [128, 65], rhs = e
    chunk [128, 512], accumulating O^T[65, l] over mc in PSUM. The 65th
    row is the softmax denominator Z (ones column of v_aug). O^T is
    DMA'd straight from PSUM to DRAM; the host divides by Z, transposes,
    and adds repeat(bv, 64). This removes ~512 LDWEIGHTS (was the PE
    bottleneck) and all DVE normalize/drain work.
  - bq is added during the qT PSUM drain (per-partition scalar on DVE);
    bk only shifts scores by a per-column constant (softmax-invariant)
    so it is dropped.

PSUM (8 banks): sA, sB [128, 3, 512] f32 (banks 0-5, exp groups),
oacc [65, 512] (bank 6, O^T accumulator), pp [128, 512] (bank 7, QKV
projection scratch).
"""

import numpy as np

import concourse.bass as bass
import concourse.mybir as mybir
import concourse.tile as tile
from concourse import bacc
from concourse.bass_utils import run_bass_kernel_spmd

B, L, D = 8, 1024, 256
H, HD = 8, 64
J = H * HD  # 512
TEMPERATURE = 10000.0

f32 = mybir.dt.float32
f16 = mybir.dt.float16

_CACHE = {}
LAST_RESULT = None
TRACE = False


def _emit(tc, aps):
    nc = tc.nc
    Exp = mybir.ActivationFunctionType.Exp
    xTd, xpd, wqd, wkd, wvd, bqd, oTd = (
        aps["xT"], aps["xpeT"], aps["wq"], aps["wk"], aps["wv"], aps["bqc"],
        aps["oT"],
    )
    xTr = xTd.rearrange("(t p) l -> t p l", p=128)    # [2, 128, 1024]
    xpr = xpd.rearrange("(t p) l -> t p l", p=128)
    wqr = wqd.rearrange("(t p) j -> t p j", p=128)    # [2, 128, 512]
    wkr = wkd.rearrange("(t p) j -> t p j", p=128)
    wvr = wvd.rearrange("(t p) j -> t p j", p=128)

    import contextlib
    ctx = contextlib.ExitStack()
    persist = ctx.enter_context(tc.tile_pool(name="persist", bufs=1))
    epool = ctx.enter_context(tc.tile_pool(name="epool", bufs=24))
    pspool = ctx.enter_context(tc.tile_pool(name="ps", bufs=1, space="PSUM"))

    # --- ACT exp-table preload (off the critical path) ---
    sc_in = persist.tile([128, 8], f32, name="sc_in")
    sc_out = persist.tile([128, 8], f32, name="sc_out")
    nc.vector.memset(sc_in[:], 0.0)
    nc.scalar.activation(sc_out[:], sc_in[:], Exp)

    # --- SBUF ---
    xpe_sb = [persist.tile([128, 1024], f16, name=f"xpe{t}") for t in range(2)]
    xT_sb = [persist.tile([128, 1024], f16, name=f"xT{t}") for t in range(2)]
    w_sb = {
        w: [persist.tile([128, 512], f16, name=f"{w}{t}") for t in range(2)]
        for w in ("wq", "wk", "wv")
    }
    bq_sb = persist.tile([128, 4], f32, name="bq_sb")
    kT = [persist.tile([128, 1024], f16, name=f"kT{u}") for u in range(4)]
    qT = [persist.tile([128, 1024], f16, name=f"qT{u}") for u in range(4)]
    v_sb = [persist.tile([128, 8, 65], f16, name=f"v{m}") for m in range(8)]

    # --- PSUM: 3+3 exp-group banks, 1 O bank, 1 projection bank ---
    sgrp = [pspool.tile([128, 3, 512], f32, name="sA")]
    pp = pspool.tile([128, 512], f32, name="pp")
    sgrp.append(pspool.tile([128, 3, 512], f32, name="sB"))
    oacc = pspool.tile([65, 512], f32, name="oacc")

    # --- input DMAs: three parallel queues (sync->Q1, gpsimd->Q0,
    # scalar->Q10, each ~75 GB/s, ~740ns issue). Critical path to the
    # first S chunk is K(0,0)+Q(0,0): wk/wq j-cols 0:128 + xpe l-half 0,
    # 384KB split three ways. Everything else queues behind. ---
    # critical wave on the two HWDGE queues (sync, scalar) — SWDGE
    # (gpsimd) transfers start late and run slow, keep it off the
    # startup path
    nc.sync.dma_start(out=xpe_sb[0][:, 0:512], in_=xpr[0][:, 0:512])
    nc.sync.dma_start(out=w_sb["wq"][0][:, 0:128], in_=wqr[0][:, 0:128])
    nc.sync.dma_start(out=w_sb["wk"][0][:, 0:128], in_=wkr[0][:, 0:128])
    nc.scalar.dma_start(out=bq_sb[:], in_=bqd[:, :])
    nc.scalar.dma_start(out=xpe_sb[1][:, 0:512], in_=xpr[1][:, 0:512])
    nc.scalar.dma_start(out=w_sb["wq"][1][:, 0:128], in_=wqr[1][:, 0:128])
    nc.scalar.dma_start(out=w_sb["wk"][1][:, 0:128], in_=wkr[1][:, 0:128])
    # second wave: xpe h1 (K/Q(0,1)), then w rests (K/Q(1..3) start
    # early in pair 0), wv+xT (V pieces run late in pair 0)
    nc.sync.dma_start(out=xpe_sb[0][:, 512:1024], in_=xpr[0][:, 512:1024])
    nc.scalar.dma_start(out=xpe_sb[1][:, 512:1024], in_=xpr[1][:, 512:1024])
    nc.sync.dma_start(out=w_sb["wk"][0][:, 128:512], in_=wkr[0][:, 128:512])
    nc.scalar.dma_start(out=w_sb["wk"][1][:, 128:512], in_=wkr[1][:, 128:512])
    nc.sync.dma_start(out=w_sb["wq"][0][:, 128:512], in_=wqr[0][:, 128:512])
    nc.scalar.dma_start(out=w_sb["wq"][1][:, 128:512], in_=wqr[1][:, 128:512])
    nc.gpsimd.dma_start(out=w_sb["wv"][0][:], in_=wvr[0])
    nc.gpsimd.dma_start(out=w_sb["wv"][1][:], in_=wvr[1])
    nc.gpsimd.dma_start(out=xT_sb[0][:], in_=xTr[0])
    nc.sync.dma_start(out=xT_sb[1][:], in_=xTr[1])
    # ones columns of v_aug (gpsimd: SBUF-only op, keeps DVE free)
    for m in range(8):
        nc.gpsimd.memset(v_sb[m][:, :, 64:65], 1.0)

    # --- projections (PSUM bank 3, DVE drains) ---
    def kq_piece(u, which, l2, scratch=None):
        dst = pp[:] if scratch is None else scratch
        wname = "wq" if which == "q" else "wk"
        for c2 in range(2):
            nc.tensor.matmul(
                dst,
                lhsT=w_sb[wname][c2][:, u * 128:(u + 1) * 128],
                rhs=xpe_sb[c2][:, l2 * 512:(l2 + 1) * 512],
                start=(c2 == 0),
                stop=(c2 == 1),
            )
        dsl = slice(l2 * 512, (l2 + 1) * 512)
        if which == "q":
            nc.vector.tensor_scalar_add(qT[u][:, dsl], dst, bq_sb[:, u:u + 1])
        else:
            nc.vector.tensor_copy(kT[u][:, dsl], dst)

    def v_piece(m):
        for c2 in range(2):
            nc.tensor.matmul(
                pp[:],
                lhsT=xT_sb[c2][:, m * 128:(m + 1) * 128],
                rhs=w_sb["wv"][c2][:],
                start=(c2 == 0),
                stop=(c2 == 1),
            )
        nc.vector.tensor_copy(
            v_sb[m][:, :, 0:64], pp[:].rearrange("p (h d) -> p h d", h=8)
        )

    # --- S chunks + grouped exp ---
    epos = {}  # (h, mc, l2) -> (e_tile, chunk_idx)
    st = {"g": 0, "c": 0, "gi": 0, "warm": 0, "keys": []}

    # exp(0.125*s) on DVE via the Schraudolph bit trick: the fp32->u16
    # convert truncates, so the +0.5 recentres; u16 wraps (not
    # saturates) below i=0, safe while |s| < 82 (data: |s|max ~ 56).
    ALPHA = float(0.125 * 1024.0 / np.log(2.0))
    BETA = float(1024.0 * (15.0 - 0.05) + 0.5)
    u16 = mybir.dt.uint16

    def flush_exp():
        n = st["c"]
        if n == 0:
            return
        e = epool.tile([128, 3, 512], f16, tag="e", name="e")
        if st["gi"] % 4 == 2 and n == 3:
            nc.vector.tensor_scalar(
                e[:, 0:n, :].bitcast(u16), sgrp[st["g"]][:, 0:n, :],
                ALPHA, BETA, mybir.AluOpType.mult, mybir.AluOpType.add,
            )
        else:
            nc.scalar.activation(
                e[:, 0:n, :], sgrp[st["g"]][:, 0:n, :], Exp,
                scale=float(HD) ** -0.5
            )
        for i, key in enumerate(st["keys"]):
            epos[key] = (e, i)
        st["g"] ^= 1
        st["gi"] += 1
        st["c"] = 0
        st["keys"] = []

    def s_chunk(h, mc, l2):
        u, half = h // 2, (h % 2) * 64
        dst = sgrp[st["g"]][:, st["c"], :]
        nc.tensor.matmul(
            dst,
            lhsT=kT[u][half:half + 64, mc * 128:(mc + 1) * 128],
            rhs=qT[u][half:half + 64, l2 * 512:(l2 + 1) * 512],
            start=True,
            stop=True,
        )
        st["keys"].append((h, mc, l2))
        st["c"] += 1
        if st["warm"] > 0:
            st["warm"] -= 1
            flush_exp()
        elif st["c"] == 3:
            flush_exp()

    # --- O: v-stationary accumulation of O^T into oacc; DVE drains to
    # SBUF (DMA has no PSUM read path), then DMA out ---
    ODMA = [nc.sync, nc.gpsimd, nc.sync, nc.gpsimd,
            nc.sync, nc.gpsimd, nc.sync, nc.gpsimd]
    opool = ctx.enter_context(tc.tile_pool(name="opool", bufs=8))

    def o_mm(h, l2, mc, acc=None):
        e, ci = epos[(h, mc, l2)]
        nc.tensor.matmul(
            oacc[:] if acc is None else acc,
            lhsT=v_sb[mc][:, h, :],
            rhs=e[:, ci, :],
            start=(mc == 0),
            stop=(mc == 7),
        )

    def o_dma(h, l2, acc=None):
        # split the write-back across two queues (three in the tail,
        # when ACT has gone idle) so the final DMAs don't serialize
        o_sb = opool.tile([65, 512], f32, tag="o", name="o_sb")
        nc.vector.tensor_copy(o_sb[:], oacc[:] if acc is None else acc)
        engs = (nc.sync, nc.scalar) if h >= 5 else (nc.sync, nc.gpsimd)
        n = len(engs)
        w = 512 // n
        for i, eng in enumerate(engs):
            sl = slice(i * w, 512 if i == n - 1 else (i + 1) * w)
            eng.dma_start(out=oTd[h, l2][:, sl], in_=o_sb[:, sl])

    # ---------------- schedule ----------------
    # startup: K and Q on different PSUM banks so their MMs/drains
    # overlap instead of serializing on one scratch bank
    kq_piece(0, "k", 0)
    kq_piece(0, "q", 0, scratch=sgrp[1][:, 0, :])
    st["warm"] = 6

    def s_order_pair(u):
        hA, hB = 2 * u, 2 * u + 1
        seq = []
        for mc in range(8):
            seq += [(hA, mc, 0), (hB, mc, 0), (hA, mc, 1), (hB, mc, 1)]
        return seq

    # Driver: emit S chunks in groups of 3 FIRST (the exp fires at the
    # 3rd chunk), then that group's share of O/proj/V work. Keeping the
    # S chunks at the head of each in-order PE queue segment keeps ACT
    # fed; the extras fill the exp window behind them.
    def run_section(chunks, extras, flush_after=True, pace=None):
        ex = list(extras)
        n = len(chunks)
        emitted = 0
        for i, chk in enumerate(chunks):
            s_chunk(*chk)
            if pace is None:
                target = int(len(extras) * (i + 1) / (n + 1))
            else:
                target = max([v for t, v in pace.items() if t <= i + 1] or [0])
            while emitted < target and ex:
                ex.pop(0)()
                emitted += 1
        if flush_after:
            flush_exp()
        while ex:
            ex.pop(0)()

    # pair 0: chunk order staged by DMA arrival: mc<4 l2q=0 needs only
    # K(0,0)+Q(0,0); l2q=1 needs Q(0,1) (xpe h1); mc>=4 needs K(0,1).
    seq0 = []
    for mc in range(4):
        seq0 += [(0, mc, 0), (1, mc, 0)]
    for mc in range(4):
        seq0 += [(0, mc, 1), (1, mc, 1)]
    for mc in range(4, 8):
        seq0 += [(0, mc, 0), (1, mc, 0)]
    for mc in range(4, 8):
        seq0 += [(0, mc, 1), (1, mc, 1)]
    # K(0,1)/Q(0,1) must precede chunk 8 (first l2q=1). V pieces are NOT
    # here: xT arrives late on the slow SWDGE queue, and a stalled V MM
    # would block the in-order PE queue; V runs in pair 1, just ahead of
    # its O consumers.
    extras0 = [
        lambda: kq_piece(0, "k", 1), lambda: kq_piece(0, "q", 1),
        lambda: kq_piece(1, "k", 0), lambda: kq_piece(1, "k", 1),
        lambda: kq_piece(1, "q", 0), lambda: kq_piece(1, "q", 1),
    ]
    run_section(seq0, extras0,
                pace={4: 2, 14: 3, 18: 4, 23: 5, 27: 6})

    # O work is allocated to sections one series later than the naive
    # split so each section's PE load stays under the ACT exp pace
    # (pair 1 also carries the 8 V pieces).
    def o_series_thunks(oh, ol2):
        t = [(lambda a, b, c: (lambda: o_mm(a, b, c)))(oh, ol2, mi) for mi in range(8)]
        t.append((lambda a, b: (lambda: o_dma(a, b)))(oh, ol2))
        return t

    def kq_thunk(u, w, l2):
        return (lambda a, b, c: (lambda: kq_piece(a, b, c)))(u, w, l2)

    # pair 1: V pieces (2 ahead of their O consumers) + O(h0) + O(h1,0)
    extras = [(lambda m: (lambda: v_piece(m)))(m) for m in (0, 1)]
    s00 = o_series_thunks(0, 0)
    for mi in range(6):
        extras.append((lambda m: (lambda: v_piece(m)))(mi + 2))
        extras.append(s00[mi])
    extras += s00[6:]
    extras += [kq_thunk(2, "k", 0), kq_thunk(2, "k", 1)]
    extras += o_series_thunks(0, 1)
    extras += [kq_thunk(2, "q", 0), kq_thunk(2, "q", 1)]
    extras += o_series_thunks(1, 0)
    run_section(s_order_pair(1), extras)

    # pair 2: O(h1,1) + O(h2) + O(h3) + K/Q(3) — pair 2 has PE slack,
    # so it absorbs all of head 3's output work, relieving pair 3
    extras = o_series_thunks(1, 1)
    extras += [kq_thunk(3, "k", 0), kq_thunk(3, "k", 1)]
    extras += o_series_thunks(2, 0)
    extras += [kq_thunk(3, "q", 0), kq_thunk(3, "q", 1)]
    extras += o_series_thunks(2, 1) + o_series_thunks(3, 0)
    run_section(s_order_pair(2), extras)

    # pair 3: head-sequential S so O(hA) can overlap S(hB)
    hA, hB = 6, 7
    seqA = [(hA, mc, l2) for mc in range(8) for l2 in range(2)]
    seqB = [(hB, mc, l2) for mc in range(8) for l2 in range(2)]
    extras = o_series_thunks(3, 1) + o_series_thunks(4, 0) + o_series_thunks(4, 1)
    run_section(seqA, extras)  # flush: hA fully exp'd before O(hA)
    ppo = pp[0:65, :]
    extras = (o_series_thunks(5, 0) + o_series_thunks(5, 1)
              + o_series_thunks(hA, 0) + o_series_thunks(hA, 1))
    # O(hB): both l2-series on separate banks, interleaved mc-by-mc so
    # only the last couple of MMs trail the final exp
    for mc in range(8):
        extras.append((lambda m: (lambda: o_mm(hB, 0, m, acc=oacc[:])))(mc))
        extras.append((lambda m: (lambda: o_mm(hB, 1, m, acc=ppo)))(mc))
    extras.append(lambda: o_dma(hB, 0, acc=oacc[:]))
    extras.append(lambda: o_dma(hB, 1, acc=ppo))
    run_section(seqB, extras)
    ctx.close()


def _build(debug=False):
    key = ("nc", debug)
    if key in _CACHE:
        return _CACHE[key]
    nc = bacc.Bacc("TRN2", target_bir_lowering=False, debug=debug, num_devices=8)
    aps = {
        "xT": nc.dram_tensor("xT", [D, L], f16, kind="ExternalInput").ap(),
        "xpeT": nc.dram_tensor("xpeT", [D, L], f16, kind="ExternalInput").ap(),
        "wq": nc.dram_tensor("wq", [D, J], f16, kind="ExternalInput").ap(),
        "wk": nc.dram_tensor("wk", [D, J], f16, kind="ExternalInput").ap(),
        "wv": nc.dram_tensor("wv", [D, J], f16, kind="ExternalInput").ap(),
        "bqc": nc.dram_tensor("bqc", [128, 4], f32, kind="ExternalInput").ap(),
        "oT": nc.dram_tensor("oT", [H, 2, 65, 512], f32, kind="ExternalOutput").ap(),
    }
    with tile.TileContext(nc) as tc:
        _emit(tc, aps)
    nc.compile()
    _CACHE[key] = nc
    return nc


def _pe():
    embed = np.arange(L, dtype=np.float32)
    dim_t = np.arange(D, dtype=np.float32)
    dim_t = (np.float32(TEMPERATURE) ** (2.0 * np.floor(dim_t / 2.0) / np.float32(D))).astype(np.float32)
    pos = embed[:, None] / dim_t
    return np.stack(
        [np.sin(pos[:, 0::2]), np.cos(pos[:, 1::2])], axis=2
    ).reshape(L, D).astype(np.float32)


def make_in_maps(inputs):
    x = np.asarray(inputs["x"], dtype=np.float32)
    wq = np.ascontiguousarray(np.asarray(inputs["Wq"], np.float32).astype(np.float16))
    wk = np.ascontiguousarray(np.asarray(inputs["Wk"], np.float32).astype(np.float16))
    wv = np.ascontiguousarray(np.asarray(inputs["Wv"], np.float32).astype(np.float16))
    bq = np.asarray(inputs["bq"], dtype=np.float32)
    bqc = np.ascontiguousarray(np.repeat(bq, HD).reshape(4, 128).T)
    pe = _pe()
    base = {"wq": wq, "wk": wk, "wv": wv, "bqc": bqc}
    in_maps = []
    for b in range(B):
        xT = np.ascontiguousarray(x[b].T.astype(np.float16))
        xpeT = np.ascontiguousarray((x[b] + pe).T.astype(np.float16))
        in_maps.append({**base, "xT": xT, "xpeT": xpeT})
    return in_maps


def finish(oT_list, bv):
    outs = []
    for oT in oT_list:  # each [8, 2, 65, 512]
        N = oT[:, :, :64, :] / oT[:, :, 64:65, :]      # [8, 2, 64, 512]
        outs.append(N.transpose(1, 3, 0, 2).reshape(L, J))
    out = np.stack(outs).astype(np.float32)
    out += np.repeat(np.asarray(bv, np.float32), HD)[None, None, :]
    return out


def kernel(**inputs):
    global LAST_RESULT
    nc = _build()
    in_maps = make_in_maps(inputs)
    res = run_bass_kernel_spmd(nc, in_maps, core_ids=list(range(B)), trace=TRACE)
    LAST_RESULT = res
    return finish([res.results[b]["oT"] for b in range(B)], inputs["bv"])
